# revision 9
# baseline (speedup 1.0000x reference)
"""Trainium2 kernel for FFT-based converged inhibition along the channel axis.

The reference computes y = IFFT(FFT(x, axis=C) / FFT(delta - k_padded)).real,
i.e. a circular convolution of each channel fiber with the fixed length-C
kernel g = IFFT(1/FFT(delta - k)).  That is a circulant matmul Y = G @ X with
G[m, c] = g[(m - c) mod C], applied independently at every (batch, h, w).

Device strategy (8 NeuronCores, data-parallel over batch):
  - each core gets 2 of the 16 batches: X_core [2, 512, 3136]
  - Y[b] = G @ X[b] as TensorE matmuls: lhsT = G^T tiles [128k, 128m],
    rhs = X k-tiles [128, 448], accumulate over k in PSUM.
  - g decays below ~1e-15 beyond +/-128 channels for this filter, so the
    k-tile at circular block distance 2 is skipped (3 of 4 k-tiles per
    output tile); verified numerically at build time, falls back to dense.
  - operands are fed as float32r (full-rate fp32 matmul mode on TRN2).
"""

import numpy as np

import concourse.bass as bass
import concourse.tile as tile
from concourse import bacc, mybir
from concourse.bass_utils import run_bass_kernel_spmd

N_CORES = 8
C = 512  # channels (FFT axis)
KT = C // 128  # 4 k/m tiles of 128 channels
FCH = 448  # free-dim chunk (fits one PSUM bank, uniform: 3136 = 7*448)

_CACHE = {}


def _build_program(n_batch_per_core: int, hw: int, band3: bool):
    """Per-core SPMD program: y[b] = G @ x[b] for n_batch_per_core batches."""
    nfc = hw // FCH
    assert nfc * FCH == hw
    nc = bacc.Bacc("TRN2", target_bir_lowering=False, debug=False)
    x_d = nc.dram_tensor(
        "x", [n_batch_per_core, C, hw], mybir.dt.float32r, kind="ExternalInput"
    ).ap()
    gt_d = nc.dram_tensor(
        "gt", [KT, 128, C], mybir.dt.float32r, kind="ExternalInput"
    ).ap()
    y_d = nc.dram_tensor(
        "y", [n_batch_per_core, C, hw], mybir.dt.float32, kind="ExternalOutput"
    ).ap()

    # first output tile m=0 needs k-tiles {3, 0, 1}
    first_ks = [3, 0, 1] if band3 else [0, 1, 2, 3]
    nfa = (nfc + 1) // 2  # chunks in first input half (4 of 7)
    ca = nfa * FCH  # split column (1792)

    # DMA plan: HWDGE lanes are assigned round-robin in scheduled order; keep
    # the order such that no input dispatch ever waits on an output-occupied
    # lane.  rhs[(b, kt, f)] = (tile, col_offset) for matmul rhs slicing.
    rhs = {}

    with tile.TileContext(nc) as tc:
        with (
            tc.tile_pool(name="gt", bufs=1) as gt_pool,
            tc.tile_pool(name="x", bufs=1) as x_pool,
            tc.tile_pool(name="ps", bufs=8, space="PSUM") as ps_pool,
            tc.tile_pool(name="out", bufs=4) as out_pool,
        ):
            # 1 DMA: all of G^T  [128, kt, m]
            gt_sb = gt_pool.tile([128, KT, C], mybir.dt.float32r, tag="gt")
            nc.sync.dma_start(
                gt_sb[:], gt_d.rearrange("kt p m -> p kt m")
            )

            def pack_in(b, c0, c1, tag):
                t = x_pool.tile([128, KT, c1 - c0], mybir.dt.float32r, tag=tag)
                nc.sync.dma_start(
                    t[:],
                    x_d[b, :, c0:c1].rearrange("(kt p) c -> p kt c", p=128),
                )
                for kt in range(KT):
                    for f in range(c0 // FCH, c1 // FCH):
                        rhs[(b, kt, f)] = (t[:, kt, :], f * FCH - c0)
                return t

            # batch 0 half A: fine-grained per k-tile for early PE start
            for kt in first_ks:
                tt = x_pool.tile([128, FCH], mybir.dt.float32r, tag=f"xf{kt}")
                nc.sync.dma_start(tt[:], x_d[0, 128 * kt : 128 * (kt + 1), :FCH])
                rhs[(0, kt, 0)] = (tt[:], 0)
            for kt in first_ks:
                tr = x_pool.tile([128, ca - FCH], mybir.dt.float32r, tag=f"xr{kt}")
                nc.sync.dma_start(
                    tr[:], x_d[0, 128 * kt : 128 * (kt + 1), FCH:ca]
                )
                for f in range(1, nfa):
                    rhs[(0, kt, f)] = (tr[:], (f - 1) * FCH)
            for kt in range(KT):
                if kt not in first_ks:
                    t2 = x_pool.tile([128, ca], mybir.dt.float32r, tag=f"xa{kt}")
                    nc.sync.dma_start(t2[:], x_d[0, 128 * kt : 128 * (kt + 1), :ca])
                    for f in range(nfa):
                        rhs[(0, kt, f)] = (t2[:], f * FCH)
            pack_in(0, ca, hw, "x0B")
            for b in range(1, n_batch_per_core):
                pack_in(b, 0, ca, f"x{b}A")
                pack_in(b, ca, hw, f"x{b}B")

            # process each batch's half-A columns (arrive first) across all m,
            # then half-B, so PE never stalls on late input halves
            for b in range(n_batch_per_core):
                for half, fr in enumerate([range(nfa), range(nfa, nfc)]):
                    c0 = 0 if half == 0 else ca
                    c1 = ca if half == 0 else hw
                    for m in range(KT):
                        if band3:
                            ks = [(m + KT - 1) % KT, m, (m + 1) % KT]
                        else:
                            ks = list(range(KT))
                        o = out_pool.tile(
                            [128, c1 - c0], mybir.dt.float32, tag=f"out{half}"
                        )
                        for f in fr:
                            ps = ps_pool.tile(
                                [128, FCH],
                                mybir.dt.float32,
                                tag="ps",
                                name=f"ps{b}_{m}_{f}",
                            )
                            for ki, kt in enumerate(ks):
                                t, off = rhs[(b, kt, f)]
                                nc.tensor.matmul(
                                    ps[:],
                                    gt_sb[:, kt, 128 * m : 128 * (m + 1)],
                                    t[:, off : off + FCH],
                                    start=(ki == 0),
                                    stop=(ki == len(ks) - 1),
                                )
                            nc.vector.tensor_copy(
                                o[:, FCH * f - c0 : FCH * (f + 1) - c0], ps[:]
                            )
                        nc.scalar.dma_start(
                            y_d[b, 128 * m : 128 * (m + 1), c0:c1], o[:]
                        )

    nc.compile()
    return nc


def _circulant_gt(inhibition_filter: np.ndarray, c: int):
    """g = IFFT(1/FFT(delta - pad_roll(k))) in float64; returns (G^T, band3_ok)."""
    scope = inhibition_filter.shape[0]
    k = np.zeros(c, np.float64)
    k[:scope] = inhibition_filter.astype(np.float64)
    k = np.roll(k, -(scope // 2))
    delta = np.zeros(c, np.float64)
    delta[0] = 1.0
    g = np.fft.ifft(1.0 / np.fft.fft(delta - k)).real
    idx = (np.arange(c)[:, None] - np.arange(c)[None, :]) % c  # G[m, cc] = g[m-cc]
    G = g[idx]
    # band check: can the k-tile at circular block distance 2 be skipped?
    dist = np.minimum(np.arange(c), c - np.arange(c))
    tail = np.abs(g[dist > 128]).max() if (dist > 128).any() else 0.0
    band3_ok = tail <= 1e-9 * np.abs(g).max()
    return np.ascontiguousarray(G.T, dtype=np.float32), band3_ok


def _reset_device():
    """Recover a wedged NeuronCore (NRT_EXEC_UNIT_UNRECOVERABLE) via axon."""
    try:
        import ctypes

        import jax

        jax.devices()
        lib = ctypes.CDLL("/opt/axon/libaxon_pjrt.so")
        if hasattr(lib, "axon_reset"):
            lib.axon_reset.restype = ctypes.c_int64
            lib.axon_reset()
    except Exception:
        pass


def kernel(activations: np.ndarray, inhibition_filter: np.ndarray) -> np.ndarray:
    return _run(activations, inhibition_filter, trace=False)[0]


def _run(activations, inhibition_filter, trace=False):
    activations = np.ascontiguousarray(activations, dtype=np.float32)
    n, c, h, w_ = activations.shape
    assert c == C and n % N_CORES == 0
    hw = h * w_
    npc = n // N_CORES

    gt, band3 = _circulant_gt(np.asarray(inhibition_filter, np.float32), c)
    gt = gt.reshape(KT, 128, C)

    key = (npc, hw, band3)
    if key not in _CACHE:
        _CACHE[key] = _build_program(npc, hw, band3)
    nc = _CACHE[key]

    xs = activations.reshape(N_CORES, npc, C, hw)
    in_maps = [{"x": xs[i], "gt": gt} for i in range(N_CORES)]
    try:
        res = run_bass_kernel_spmd(nc, in_maps, list(range(N_CORES)), trace=trace)
    except Exception:
        _reset_device()
        res = run_bass_kernel_spmd(nc, in_maps, list(range(N_CORES)), trace=trace)
    y = np.stack([res.results[i]["y"] for i in range(N_CORES)])
    y = y.reshape(n, c, h, w_).astype(np.float32, copy=False)
    return y, res


# revision 16
# speedup vs baseline: 1.2538x; 1.2538x over previous
"""Trainium2 kernel for FFT-based converged inhibition along the channel axis.

The reference computes y = IFFT(FFT(x, axis=C) / FFT(delta - k_padded)).real,
i.e. a circular convolution of each channel fiber with the fixed length-C
kernel g = IFFT(1/FFT(delta - k)).  That is a circulant matmul Y = G @ X with
G[m, c] = g[(m - c) mod C], applied independently at every (batch, h, w).

Device strategy (8 NeuronCores, data-parallel over batch):
  - each core gets 2 of the 16 batches: X_core [2, 512, 3136]
  - Y[b] = G @ X[b] as TensorE matmuls: lhsT = G^T tiles [128k, 128m],
    rhs = X k-tiles [128, 448], accumulate over k in PSUM.
  - g decays below ~1e-15 beyond +/-128 channels for this filter, so the
    k-tile at circular block distance 2 is skipped (3 of 4 k-tiles per
    output tile); verified numerically at build time, falls back to dense.
  - operands are fed as float32r (full-rate fp32 matmul mode on TRN2).
"""

import numpy as np

import concourse.bass as bass
import concourse.tile as tile
from concourse import bacc, mybir
from concourse.bass_utils import run_bass_kernel_spmd

N_CORES = 8
C = 512  # channels (FFT axis)
KT = C // 128  # 4 k/m tiles of 128 channels
FCH = 448  # free-dim chunk (fits one PSUM bank, uniform: 3136 = 7*448)

_CACHE = {}


def _build_program(n_batch_per_core: int, hw: int, band3: bool):
    """Per-core SPMD program: y[b] = G @ x[b] for n_batch_per_core batches."""
    nfc = hw // FCH
    assert nfc * FCH == hw
    nc = bacc.Bacc(
        "TRN2", target_bir_lowering=False, debug=False, enable_asserts=False
    )
    x_d = nc.dram_tensor(
        "x", [n_batch_per_core, C, hw], mybir.dt.float32r, kind="ExternalInput"
    ).ap()
    gt_d = nc.dram_tensor(
        "gt", [KT, 128, C], mybir.dt.float32r, kind="ExternalInput"
    ).ap()
    y_d = nc.dram_tensor(
        "y", [n_batch_per_core, C, hw], mybir.dt.float32, kind="ExternalOutput"
    ).ap()

    # first output tile m=0 needs k-tiles {3, 0, 1}
    first_ks = [3, 0, 1] if band3 else [0, 1, 2, 3]
    nfa = (nfc + 1) // 2  # chunks in first input half (4 of 7)
    ca = nfa * FCH  # split column (1792)

    # DMA plan: HWDGE lanes are assigned round-robin in scheduled order; keep
    # the order such that no input dispatch ever waits on an output-occupied
    # lane.  rhs[(b, kt, f)] = (tile, col_offset) for matmul rhs slicing.
    rhs = {}

    with tile.TileContext(nc) as tc:
        with (
            tc.tile_pool(name="gt", bufs=1) as gt_pool,
            tc.tile_pool(name="x", bufs=1) as x_pool,
            tc.tile_pool(name="ps", bufs=8, space="PSUM") as ps_pool,
            tc.tile_pool(name="out", bufs=4) as out_pool,
        ):
            # 1 DMA: all of G^T  [128, kt, m]
            gt_sb = gt_pool.tile([128, KT, C], mybir.dt.float32r, tag="gt")
            nc.sync.dma_start(gt_sb[:], gt_d.rearrange("kt p m -> p kt m"))

            def in_dma(b, kt, c0, c1, tag):
                t = x_pool.tile([128, c1 - c0], mybir.dt.float32r, tag=tag)
                nc.sync.dma_start(t[:], x_d[b, 128 * kt : 128 * (kt + 1), c0:c1])
                for f in range(c0 // FCH, c1 // FCH):
                    rhs[(b, kt, f)] = (t[:], f * FCH - c0)
                return t

            # Inputs in rounds of <=8 DMAs.  HWDGE lanes are assigned
            # round-robin over 8 in scheduled order and each lane is FIFO, so
            # round r+1 transfers only start as round r completes: sections
            # complete in consumption order and all inputs run before outputs.
            kseq = first_ks + [kt for kt in range(KT) if kt not in first_ks]
            # round 1 (with gt): b0 half A, f0 chunks first for early PE start
            for kt in first_ks:
                tt = x_pool.tile([128, FCH], mybir.dt.float32r, tag=f"xf{kt}")
                nc.sync.dma_start(tt[:], x_d[0, 128 * kt : 128 * (kt + 1), :FCH])
                rhs[(0, kt, 0)] = (tt[:], 0)
            for kt in first_ks:
                tr = x_pool.tile([128, ca - FCH], mybir.dt.float32r, tag=f"xr{kt}")
                nc.sync.dma_start(tr[:], x_d[0, 128 * kt : 128 * (kt + 1), FCH:ca])
                for f in range(1, nfa):
                    rhs[(0, kt, f)] = (tr[:], (f - 1) * FCH)
            for kt in range(KT):
                if kt not in first_ks:
                    in_dma(0, kt, 0, ca, f"xa{kt}")
            # round 2: b0 half B then b1 half A; round 3: b1 half B
            for kt in kseq:
                in_dma(0, kt, ca, hw, f"xb{kt}")
            for b in range(1, n_batch_per_core):
                for kt in kseq:
                    in_dma(b, kt, 0, ca, f"xc{b}_{kt}")
                for kt in kseq:
                    in_dma(b, kt, ca, hw, f"xd{b}_{kt}")

            # process each batch's half-A columns (arrive first) across all m,
            # then half-B, so PE never stalls on late input halves
            for b in range(n_batch_per_core):
                for half, fr in enumerate([range(nfa), range(nfa, nfc)]):
                    c0 = 0 if half == 0 else ca
                    c1 = ca if half == 0 else hw
                    for m in range(KT):
                        if band3:
                            ks = [(m + KT - 1) % KT, m, (m + 1) % KT]
                        else:
                            ks = list(range(KT))
                        o = out_pool.tile(
                            [128, c1 - c0], mybir.dt.float32, tag=f"out{half}"
                        )
                        for f in fr:
                            ps = ps_pool.tile(
                                [128, FCH],
                                mybir.dt.float32,
                                tag="ps",
                                name=f"ps{b}_{m}_{f}",
                            )
                            for ki, kt in enumerate(ks):
                                t, off = rhs[(b, kt, f)]
                                nc.tensor.matmul(
                                    ps[:],
                                    gt_sb[:, kt, 128 * m : 128 * (m + 1)],
                                    t[:, off : off + FCH],
                                    start=(ki == 0),
                                    stop=(ki == len(ks) - 1),
                                )
                            dst = o[:, FCH * f - c0 : FCH * (f + 1) - c0]
                            if f % 2 == 0:
                                nc.vector.tensor_copy(dst, ps[:])
                            else:
                                nc.scalar.mul(dst, ps[:], 1.0)
                        nc.scalar.dma_start(
                            y_d[b, 128 * m : 128 * (m + 1), c0:c1], o[:]
                        )

    # Strip the unused const-tile memsets Bass emits in its preamble: they
    # pull the gpsimd ucode library load into the critical entry barrier
    # (~2-8us of NEFF time) and nothing in this kernel reads them.
    for blk in nc.main_func.blocks:
        blk.instructions[:] = [
            inst
            for inst in blk.instructions
            if not (
                isinstance(inst, mybir.InstMemset)
                and inst.outs
                and "const-" in str(inst.outs[0])
            )
        ]
    nc.compile()
    return nc


def _circulant_gt(inhibition_filter: np.ndarray, c: int):
    """g = IFFT(1/FFT(delta - pad_roll(k))) in float64; returns (G^T, band3_ok)."""
    scope = inhibition_filter.shape[0]
    k = np.zeros(c, np.float64)
    k[:scope] = inhibition_filter.astype(np.float64)
    k = np.roll(k, -(scope // 2))
    delta = np.zeros(c, np.float64)
    delta[0] = 1.0
    g = np.fft.ifft(1.0 / np.fft.fft(delta - k)).real
    idx = (np.arange(c)[:, None] - np.arange(c)[None, :]) % c  # G[m, cc] = g[m-cc]
    G = g[idx]
    # band check: can the k-tile at circular block distance 2 be skipped?
    dist = np.minimum(np.arange(c), c - np.arange(c))
    tail = np.abs(g[dist > 128]).max() if (dist > 128).any() else 0.0
    band3_ok = tail <= 1e-9 * np.abs(g).max()
    return np.ascontiguousarray(G.T, dtype=np.float32), band3_ok


def _reset_device():
    """Recover a wedged NeuronCore (NRT_EXEC_UNIT_UNRECOVERABLE) via axon."""
    try:
        import ctypes

        import jax

        jax.devices()
        lib = ctypes.CDLL("/opt/axon/libaxon_pjrt.so")
        if hasattr(lib, "axon_reset"):
            lib.axon_reset.restype = ctypes.c_int64
            lib.axon_reset()
    except Exception:
        pass


def kernel(activations: np.ndarray, inhibition_filter: np.ndarray) -> np.ndarray:
    return _run(activations, inhibition_filter, trace=False)[0]


def _run(activations, inhibition_filter, trace=False):
    activations = np.ascontiguousarray(activations, dtype=np.float32)
    n, c, h, w_ = activations.shape
    assert c == C and n % N_CORES == 0
    hw = h * w_
    npc = n // N_CORES

    gt, band3 = _circulant_gt(np.asarray(inhibition_filter, np.float32), c)
    gt = gt.reshape(KT, 128, C)

    key = (npc, hw, band3)
    if key not in _CACHE:
        _CACHE[key] = _build_program(npc, hw, band3)
    nc = _CACHE[key]

    xs = activations.reshape(N_CORES, npc, C, hw)
    in_maps = [{"x": xs[i], "gt": gt} for i in range(N_CORES)]
    try:
        res = run_bass_kernel_spmd(nc, in_maps, list(range(N_CORES)), trace=trace)
    except Exception:
        _reset_device()
        res = run_bass_kernel_spmd(nc, in_maps, list(range(N_CORES)), trace=trace)
    y = np.stack([res.results[i]["y"] for i in range(N_CORES)])
    y = y.reshape(n, c, h, w_).astype(np.float32, copy=False)
    return y, res
